# revision 3
# baseline (speedup 1.0000x reference)
"""Single-token-decode attention (b=16, h=32, d=128, kv=4096) on 8 NeuronCores.

Sharding: tensor-parallel over heads — 4 heads per core. Each core computes
q/k/v projections for its heads, attention over the full KV cache slice, and
its partial x @ wo contribution; the host sums the 8 partials.

Quantized-cache design (per-core HBM traffic ~75 MiB, vs 144 MiB bf16):
  - V cache: fp8 e3m4, natural [seq, hd] layout, consumed by PE (AV matmul
    streams fp8 moving against bf16 P stationary).
  - K cache, heads 0-1 ("DVE heads"): int8 with per-row fp32 scales
    (amax/127, 1/sqrt(d) folded in). Scores via DVE scalar_tensor_tensor
    with an IMMEDIATE scalar (AP scalars cost +62 ns/op); the row scales
    are applied afterwards as one [128, NT] tensor_mul per (b, ch, head).
  - K cache, heads 2-3 ("PE heads"): fp8 e3m4, host-transposed to
    [d, seq'] with seq' permuted so the PE score matmul (K^T-tile
    stationary [128d x 128seq], q column moving) lands S in the same
    [128(p), NT] seq-partition layout the DVE heads and V tiles use
    (s_local = p*16 + t).
  - wq/wo bf16; wk/wv fp8 e3m4 with x64 host prescale (they only affect
    the single patched token).

DMA packing: the four per-(b, chunk) streams (K-int8, scales, K^T-fp8,
V-fp8) are host-packed into ONE blob with 16512 contiguous bytes per
partition, loaded with a single DMA and consumed through bitcast views —
small/strided loads were costing ~25% of effective HBM bandwidth.
Weights are host-packed to [8, 128, 2048] so each partition is one
contiguous run. Exp runs on ACT (SBUF for DVE heads, PSUM for PE heads)
with per-chunk row-sum accumulation; normalization is one reciprocal +
broadcast-matmul at the end. The stale seq-4095 row is patched on-device
with dtype-converting ACT copies DMA'd into the blob views.
"""

import ml_dtypes
import numpy as np

import concourse.bass as bass
import concourse.mybir as mybir
import concourse.tile as tile
from concourse import bacc
from concourse.bass_utils import run_bass_kernel_spmd

N_CORES = 8
B = 16          # batch
H = 4           # heads per core
NHD = 2         # DVE (int8) heads: local 0,1
NHP = 2         # PE (fp8 K^T) heads: local 2,3
D = 128         # head dim
HD = H * D      # 512
DIM = 4096
SEQ = 4096
CH = 2048       # seq chunk per round
NT = CH // 128  # seq tiles per chunk (16)
NCH = SEQ // CH  # 2
NPAIR = H * B   # (head, batch) pairs per core
SCALE = float(1.0 / np.sqrt(np.float32(D)))
KPATCH_C = 8.0  # fixed int8 scale for the on-device patched K row
WKV_PRE = 64.0  # host prescale on wk/wv so fp8 e3m4 stays in normal range
F32 = mybir.dt.float32
BF16 = mybir.dt.bfloat16
FP8 = mybir.dt.float8e3
I8 = mybir.dt.int8

# blob byte offsets (per partition)
OFF_K8 = 0                      # int8 [NT, 256]            -> 4096 B
OFF_KS = 4096                   # f32  [NT, 2]              -> 128 B
OFF_KTP = 4224                  # fp8  [2, 2048]            -> 4096 B
OFF_V = 8320                    # fp8  [NT, 512]            -> 8192 B
BLOB_B = 16512

_nc_cache = {}


def _build_nc():
    if "nc" in _nc_cache:
        return _nc_cache["nc"]
    nc = bacc.Bacc("TRN2", target_bir_lowering=False, debug=False,
                   num_devices=N_CORES)

    xT = nc.dram_tensor("xT", [128, DIM // 128, B], BF16, kind="ExternalInput").ap()
    wq = nc.dram_tensor("wq", [8, 128, 2048], BF16, kind="ExternalInput").ap()
    wk = nc.dram_tensor("wk", [8, 128, 2048], FP8, kind="ExternalInput").ap()
    wv = nc.dram_tensor("wv", [8, 128, 2048], FP8, kind="ExternalInput").ap()
    wo = nc.dram_tensor("wo", [8, 128, 2048], BF16, kind="ExternalInput").ap()
    blob = nc.dram_tensor("blob", [B, NCH, 128, BLOB_B], I8,
                          kind="ExternalInput").ap()
    cos16 = nc.dram_tensor("cos16", [B, D // 2], F32, kind="ExternalInput").ap()
    id4 = nc.dram_tensor("id4", [H, H], F32, kind="ExternalInput").ap()
    id16 = nc.dram_tensor("id16", [B, B], F32, kind="ExternalInput").ap()
    bcsel = nc.dram_tensor("bcsel", [B, B * 128], BF16,
                           kind="ExternalInput").ap()
    sin16 = nc.dram_tensor("sin16", [B, D // 2], F32, kind="ExternalInput").ap()
    out = nc.dram_tensor("out", [B, DIM], F32, kind="ExternalOutput").ap()

    batch_order = list(range(1, B)) + [0]
    first_b = batch_order[0]

    def load_blob(pool, b, ch):
        t = pool.tile([128, BLOB_B], I8)
        nc.sync.dma_start(out=t, in_=blob[b, ch])
        return t

    def blob_views(bl):
        kt = bl[:, OFF_K8:OFF_KS].rearrange("p (t n) -> p t n", t=NT)
        kst = bl[:, OFF_KS:OFF_KTP].bitcast(F32).rearrange(
            "p (t h) -> p t h", t=NT)
        ktpt = bl[:, OFF_KTP:OFF_V].bitcast(FP8).rearrange(
            "p (h c) -> p h c", h=NHP)
        vt = bl[:, OFF_V:BLOB_B].bitcast(FP8).rearrange(
            "p (t n) -> p t n", t=NT)
        return kt, kst, ktpt, vt

    def rotary(dst, src, t1, t2, cos_sb, sin_sb):
        rv = dst.rearrange("b (h i two) -> b h i two", h=H, two=2)
        sv = src.rearrange("b (h i two) -> b h i two", h=H, two=2)
        for h in range(H):
            e, o = sv[:, h, :, 0], sv[:, h, :, 1]
            nc.vector.tensor_mul(t1[:, h, :], e, cos_sb)
            nc.vector.tensor_mul(t2[:, h, :], o, sin_sb)
            nc.vector.tensor_sub(rv[:, h, :, 0], t1[:, h, :], t2[:, h, :])
            nc.vector.tensor_mul(t1[:, h, :], e, sin_sb)
            nc.vector.tensor_mul(t2[:, h, :], o, cos_sb)
            nc.vector.tensor_add(rv[:, h, :, 1], t1[:, h, :], t2[:, h, :])

    with tile.TileContext(nc) as tc:
        with (
            tc.tile_pool(name="singles", bufs=1) as singles,
            tc.tile_pool(name="blobpool", bufs=4) as blobpool,
            tc.tile_pool(name="spool", bufs=4) as spool,
            tc.tile_pool(name="ppool", bufs=4) as ppool,
            tc.tile_pool(name="scrp", bufs=2) as scrp,
            tc.tile_pool(name="rowpool", bufs=8) as rowpool,
            tc.tile_pool(name="ps_a", bufs=2, space="PSUM") as ps_a,
            tc.tile_pool(name="ps_av", bufs=2, space="PSUM") as ps_av,
            tc.tile_pool(name="ps_s", bufs=2, space="PSUM") as ps_s,
            tc.tile_pool(name="ps_tr", bufs=1, space="PSUM") as ps_tr,
            tc.tile_pool(name="ps_sum", bufs=1, space="PSUM") as ps_sum,
        ):
            with tc.tile_pool(name="wqpool", bufs=7) as wqpool, \
                 tc.tile_pool(name="wpool", bufs=2) as wpool:
                with nc.named_scope("p1_proj"):
                    # ---- tiny constants first ----
                    xT_sb = singles.tile([128, DIM // 128, B], BF16)
                    nc.sync.dma_start(out=xT_sb, in_=xT)
                    cos_sb = singles.tile([B, D // 2], F32)
                    nc.sync.dma_start(out=cos_sb, in_=cos16)
                    sin_sb = singles.tile([B, D // 2], F32)
                    nc.sync.dma_start(out=sin_sb, in_=sin16)
                    ones_sb = singles.tile([128, 1], F32)
                    nc.vector.memset(ones_sb, 1.0)
                    ones_row = singles.tile([1, 128], F32)
                    nc.vector.memset(ones_row, 1.0)
                    id4_sb = singles.tile([H, H], F32)
                    nc.sync.dma_start(out=id4_sb, in_=id4)
                    id16_sb = singles.tile([B, B], F32)
                    nc.sync.dma_start(out=id16_sb, in_=id16)
                    bcsel_sb = singles.tile([B, B * 128], BF16)
                    nc.sync.dma_start(out=bcsel_sb, in_=bcsel)

                    # first batch's chunk tensors before the weights so the
                    # S stream can start as early as possible
                    pre_bl0 = load_blob(blobpool, first_b, 0)

                    # ---- q projection (needs only wq) ----
                    qrow_sb = singles.tile([B, HD], F32)
                    proj_ps = ps_a.tile([B, HD], F32, name="proj_q",
                                        tag="work")
                    for ci in range(8):
                        wt = wqpool.tile([128, 2048], BF16, tag="w")
                        nc.sync.dma_start(out=wt, in_=wq[ci])
                        wt_v = wt.rearrange("p (s n) -> p s n", s=4)
                        for s in range(4):
                            ktile = ci * 4 + s
                            nc.tensor.matmul(proj_ps, xT_sb[:, ktile, :],
                                             wt_v[:, s, :], start=(ktile == 0),
                                             stop=(ktile == 31))
                    nc.scalar.copy(out=qrow_sb, in_=proj_ps)

                    # q rotary, then a bf16 copy for the broadcast matmuls
                    qrot_sb = singles.tile([B, HD], F32, name="rot_q")
                    t1 = singles.tile([B, H, D // 2], F32, name="t1")
                    t2 = singles.tile([B, H, D // 2], F32, name="t2")
                    rotary(qrot_sb, qrow_sb, t1, t2, cos_sb, sin_sb)
                    qrot_bf = singles.tile([B, HD], BF16, name="qrot_bf")
                    nc.scalar.copy(out=qrot_bf, in_=qrot_sb)

                    # DVE-head q broadcasts via one-hot selector matmuls
                    qbc_all = []
                    for b in range(B):
                        qbc_ps = ps_a.tile([128, NHD * D], F32, name="qbc_ps",
                                           tag="work")
                        nc.tensor.matmul(qbc_ps,
                                         bcsel_sb[:, b * 128:(b + 1) * 128],
                                         qrot_bf[:, :NHD * D],
                                         start=True, stop=True)
                        qbc_sb = singles.tile([128, NHD * D], BF16,
                                              name=f"qbc{b}")
                        nc.scalar.copy(out=qbc_sb, in_=qbc_ps)
                        qbc_all.append(qbc_sb)

                    # PE-head q columns: transpose qrot rows -> [d, b] columns
                    qT_sb = singles.tile([128, NHP, B], BF16, name="qT_sb")
                    for i in range(NHP):
                        h = NHD + i
                        qT_ps = ps_a.tile([128, B], F32, name="qT_ps",
                                          tag="work")
                        nc.tensor.transpose(qT_ps,
                                            qrot_sb[:, h * D:(h + 1) * D],
                                            id16_sb)
                        nc.scalar.copy(out=qT_sb[:, i, :], in_=qT_ps)

                    # first batch's chunk-1 before the k/v weights
                    pre_bl1 = load_blob(blobpool, first_b, 1)

                    # ---- k/v projections (fp8 weights, x64 prescaled) ----
                    krow_sb = singles.tile([B, HD], F32)
                    vnew_sb = singles.tile([B, HD], F32)
                    for w_d, row_sb in ((wk, krow_sb), (wv, vnew_sb)):
                        proj_ps = ps_a.tile([B, HD], F32, name="proj_kv",
                                            tag="work")
                        for ci in range(8):
                            wt = wpool.tile([128, 2048], FP8, tag="w")
                            nc.sync.dma_start(out=wt, in_=w_d[ci])
                            wt_v = wt.rearrange("p (s n) -> p s n", s=4)
                            for s in range(4):
                                ktile = ci * 4 + s
                                nc.tensor.matmul(proj_ps, xT_sb[:, ktile, :],
                                                 wt_v[:, s, :],
                                                 start=(ktile == 0),
                                                 stop=(ktile == 31))
                        nc.scalar.mul(out=row_sb, in_=proj_ps,
                                      mul=1.0 / WKV_PRE)

                    # k rotary + patch rows in each cache dtype
                    krot_sb = singles.tile([B, HD], F32, name="rot_k")
                    rotary(krot_sb, krow_sb, t1, t2, cos_sb, sin_sb)
                    kpatch8 = singles.tile([B, NHD * D], I8, name="kpatch8")
                    nc.scalar.mul(out=kpatch8, in_=krot_sb[:, :NHD * D],
                                  mul=127.0 / KPATCH_C)
                    kTp_sb = singles.tile([128, NHP, B], FP8, name="kTp_sb")
                    for i in range(NHP):
                        h = NHD + i
                        kT_ps = ps_a.tile([128, B], F32, name="kT_ps",
                                          tag="work")
                        nc.tensor.transpose(kT_ps,
                                            krot_sb[:, h * D:(h + 1) * D],
                                            id16_sb)
                        nc.scalar.copy(out=kTp_sb[:, i, :], in_=kT_ps)
                    vpatch8 = singles.tile([B, HD], FP8, name="vpatch8")
                    nc.scalar.copy(out=vpatch8, in_=vnew_sb)

            with tc.tile_pool(name="outp", bufs=2) as outp:
                # ---- phase 2: attention ----
                O_sb = singles.tile([128, NPAIR], F32)
                sums2_ps = ps_sum.tile([1, NCH * NPAIR], F32)
                wot_tiles = [None] * 8

                for bi, b in enumerate(batch_order):
                  with nc.named_scope(f"p2_b{b:02d}"):
                    qbc_sb = qbc_all[b]
                    av_ps = ps_av.tile([H, 512], F32)
                    rows_pair = [rowpool.tile([128, NCH], F32,
                                              name="rows_pair")
                                 for _ in range(H)]
                    for ch in range(NCH):
                        if b == first_b:
                            bl = pre_bl0 if ch == 0 else pre_bl1
                        else:
                            bl = load_blob(blobpool, b, ch)
                        kt, kst, ktpt, vt = blob_views(bl)
                        if ch == NCH - 1:
                            # seq position 4095 holds stale cache: replace
                            # with the new token's rotated k / v rows
                            nc.scalar.dma_start(out=kt[127:128, NT - 1, :],
                                                in_=kpatch8[b:b + 1, :])
                            nc.scalar.dma_start(
                                out=ktpt[:, :, CH - 1:CH],
                                in_=kTp_sb[:, :, b:b + 1])
                            nc.scalar.dma_start(out=vt[127:128, NT - 1, :],
                                                in_=vpatch8[b:b + 1, :])

                        ptil = ppool.tile([128, NT, H], BF16)
                        # DVE heads: int8 STT (immediate scalar), then one
                        # row-scale multiply
                        for hi in range(NHD):
                            s_sb = spool.tile([128, NT], F32, name="s_sb")
                            for t in range(NT):
                                scr = scrp.tile([128, D], BF16, name="scr")
                                nc.vector.scalar_tensor_tensor(
                                    out=scr,
                                    in0=kt[:, t, hi * D:(hi + 1) * D],
                                    scalar=1.0,
                                    in1=qbc_sb[:, hi * D:(hi + 1) * D],
                                    op0=mybir.AluOpType.mult,
                                    op1=mybir.AluOpType.mult,
                                    accum_out=s_sb[:, t:t + 1])
                            nc.vector.tensor_mul(s_sb, s_sb, kst[:, :, hi])
                            nc.scalar.activation(
                                out=ptil[:, :, hi], in_=s_sb,
                                func=mybir.ActivationFunctionType.Exp,
                                scale=1.0,
                                accum_out=rows_pair[hi][:, ch:ch + 1])
                        # PE heads: K^T-tile stationary x q column moving
                        s_ps = ps_s.tile([128, NT, NHP], F32, name="s_pe")
                        for i in range(NHP):
                            for t in range(NT):
                                nc.tensor.matmul(
                                    s_ps[:, t, i:i + 1],
                                    ktpt[:, i, t * 128:(t + 1) * 128],
                                    qT_sb[:, i, b:b + 1],
                                    start=True, stop=True)
                            hi = NHD + i
                            nc.scalar.activation(
                                out=ptil[:, :, hi], in_=s_ps[:, :, i],
                                func=mybir.ActivationFunctionType.Exp,
                                scale=SCALE,
                                accum_out=rows_pair[hi][:, ch:ch + 1])
                        for t in range(NT):
                            nc.tensor.matmul(av_ps, ptil[:, t, :],
                                             vt[:, t, :],
                                             start=(ch == 0 and t == 0),
                                             stop=(ch == NCH - 1
                                                   and t == NT - 1))

                    if bi < 8:
                        # spread the wo prefetch through the stream
                        wot = singles.tile([128, 2048], BF16,
                                           name=f"wot{bi}")
                        nc.sync.dma_start(out=wot, in_=wo[bi])
                        wot_tiles[bi] = wot

                    for hi in range(H):
                        pc = hi * B + b
                        nc.tensor.matmul(
                            sums2_ps[0:1, NCH * pc:NCH * pc + NCH],
                            ones_sb, rows_pair[hi], start=True, stop=True)

                    # extract block-diagonal strips of av_ps and transpose
                    # into O^T [d, pair] layout
                    av_sb = outp.tile([H, 512], F32, name="av_sb")
                    nc.scalar.copy(out=av_sb, in_=av_ps)
                    tr_ps = ps_tr.tile([128, H, H], F32)
                    for hi in range(H):
                        nc.tensor.transpose(tr_ps[:, hi, :],
                                            av_sb[:, hi * D:(hi + 1) * D],
                                            id4_sb)
                    ocols = O_sb.rearrange("p (h bb) -> p h bb", h=H)
                    for hi in range(H):
                        nc.scalar.copy(out=ocols[:, hi, b:b + 1],
                                       in_=tr_ps[:, hi, hi:hi + 1])

                # ---- phase 3: normalize + wo ----
                with nc.named_scope("p3_out"):
                    sums2_sb = singles.tile([1, NCH * NPAIR], F32)
                    nc.scalar.copy(out=sums2_sb, in_=sums2_ps)
                    totals_sb = singles.tile([1, NPAIR], F32)
                    nc.vector.tensor_reduce(
                        out=totals_sb,
                        in_=sums2_sb.rearrange("p (a b) -> p a b", b=NCH),
                        axis=mybir.AxisListType.X, op=mybir.AluOpType.add)
                    rsum_sb = singles.tile([1, NPAIR], F32)
                    nc.vector.reciprocal(out=rsum_sb, in_=totals_sb)
                    bc_ps = ps_a.tile([128, NPAIR], F32, name="bc_ps",
                                      tag="work")
                    nc.tensor.matmul(bc_ps, ones_row, rsum_sb, start=True,
                                     stop=True)
                    on_sb = singles.tile([128, NPAIR], F32)
                    nc.vector.tensor_mul(on_sb, O_sb, bc_ps)
                    on_bf = singles.tile([128, NPAIR], BF16)
                    nc.scalar.copy(out=on_bf, in_=on_sb)

                    for nch in range(8):
                        wot_v = wot_tiles[nch].rearrange(
                            "p (k j) -> p k j", k=4)
                        wo_ps = ps_a.tile([B, 512], F32, name="wo_ps",
                                          tag="work")
                        for k in range(H):
                            nc.tensor.matmul(wo_ps,
                                             on_bf[:, k * B:(k + 1) * B],
                                             wot_v[:, k, :], start=(k == 0),
                                             stop=(k == H - 1))
                        wout_sb = outp.tile([B, 512], F32, name="wout_sb")
                        nc.scalar.copy(out=wout_sb, in_=wo_ps)
                        nc.scalar.dma_start(
                            out=out[:, nch * 512:(nch + 1) * 512],
                            in_=wout_sb)

    nc.compile()
    _nc_cache["nc"] = nc
    return nc


def _pack_w_in(w, cs):
    """[4096, 512] column slice -> [8, 128, 2048] with contiguous partitions."""
    return np.ascontiguousarray(
        w[:, cs].reshape(8, 4, 128, 512).transpose(0, 2, 1, 3).reshape(
            8, 128, 2048))


def _host_prep(x, wq, wk, wv, wo, cache_k, cache_v, freqs_cos, freqs_sin):
    f32 = np.float32
    bf = ml_dtypes.bfloat16
    f8 = ml_dtypes.float8_e3m4
    xT = np.ascontiguousarray(
        x.reshape(B, DIM // 128, 128).transpose(2, 1, 0)).astype(bf)

    cos = np.asarray(freqs_cos, dtype=f32).reshape(D // 2)
    sin = np.asarray(freqs_sin, dtype=f32).reshape(D // 2)
    cos16 = np.ascontiguousarray(np.broadcast_to(cos, (B, D // 2)), dtype=f32)
    sin16 = np.ascontiguousarray(np.broadcast_to(sin, (B, D // 2)), dtype=f32)

    bcsel = np.zeros((B, B * 128), dtype=bf)
    for b in range(B):
        bcsel[b, b * 128:(b + 1) * 128] = 1.0

    kc = np.asarray(cache_k, dtype=f32)   # [B, SEQ, 32, 128]
    vc = np.asarray(cache_v, dtype=f32)
    wq_bf = np.asarray(wq, dtype=f32).astype(bf)
    wk_f8 = (np.asarray(wk, dtype=f32) * WKV_PRE).astype(f8)
    wv_f8 = (np.asarray(wv, dtype=f32) * WKV_PRE).astype(f8)
    wo_bf = np.asarray(wo, dtype=f32).astype(bf)

    in_maps = []
    for c in range(N_CORES):
        hs = slice(H * c, H * (c + 1))
        cs = slice(HD * c, HD * (c + 1))
        k_c = kc[:, :, hs, :]              # [B, SEQ, 4, 128]
        v_c = vc[:, :, hs, :]

        # --- DVE heads: int8 + per-row scales ---
        kd = k_c[:, :, :NHD, :]
        amax = np.abs(kd).max(axis=-1)
        amax = np.maximum(amax, 1e-12)
        kq8 = np.clip(np.rint(kd * (127.0 / amax[..., None])),
                      -127, 127).astype(np.int8)
        # scales: [B, NCH, 128(p), NT(t), 2]; fold 1/sqrt(d)
        sc = (amax / (127.0 * np.sqrt(np.float32(D)))).astype(f32)
        sc = sc.reshape(B, NCH, 128, NT, NHD)
        sc[:, 1, 127, 15, :] = KPATCH_C / (127.0 * np.sqrt(np.float32(D)))

        # --- PE heads: fp8 K^T with seq' = (t, p) permuted columns ---
        kp = k_c[:, :, NHD:, :]
        kp = kp.reshape(B, NCH, 128, NT, NHP, D)
        kp = kp.transpose(0, 1, 5, 4, 3, 2)     # b, ch, d(p), h, t, p
        # per-partition layout [h, c=t*128+p]: need [b, ch, d, h, t, p]
        ktp_bytes = np.ascontiguousarray(
            kp.astype(f8)).view(np.int8).reshape(B, NCH, 128, 4096)

        k8_bytes = kq8.reshape(B, NCH, 128, NT, NHD * D).reshape(
            B, NCH, 128, 4096)
        ks_bytes = np.ascontiguousarray(sc).view(np.int8).reshape(
            B, NCH, 128, 128)
        v8_bytes = np.ascontiguousarray(
            v_c.reshape(B, NCH, 128, NT, HD).astype(f8)).view(
                np.int8).reshape(B, NCH, 128, 8192)

        blob = np.concatenate([k8_bytes, ks_bytes, ktp_bytes, v8_bytes],
                              axis=-1)
        assert blob.shape[-1] == BLOB_B

        in_maps.append({
            "xT": xT,
            "id4": np.eye(H, dtype=f32),
            "id16": np.eye(B, dtype=f32),
            "bcsel": bcsel,
            "wq": _pack_w_in(wq_bf, cs),
            "wk": _pack_w_in(wk_f8, cs),
            "wv": _pack_w_in(wv_f8, cs),
            "wo": np.ascontiguousarray(
                wo_bf[cs, :].reshape(4, 128, 8, 512).transpose(
                    2, 1, 0, 3).reshape(8, 128, 2048)),
            "blob": blob,
            "cos16": cos16,
            "sin16": sin16,
        })
    return in_maps


def kernel(x, wq, wk, wv, wo, cache_k, cache_v, freqs_cos, freqs_sin,
           start_pos, _trace=False, _trace_kwargs=None):
    assert int(start_pos) == SEQ - 1, "kernel is specialized for start_pos=4095"
    in_maps = _host_prep(np.asarray(x, dtype=np.float32), np.asarray(wq),
                         np.asarray(wk), np.asarray(wv), np.asarray(wo),
                         np.asarray(cache_k), np.asarray(cache_v),
                         np.asarray(freqs_cos), np.asarray(freqs_sin))
    nc = _build_nc()
    kwargs = {}
    if _trace:
        kwargs["trace"] = True
        if _trace_kwargs:
            kwargs.update(_trace_kwargs)
    res = run_bass_kernel_spmd(nc, in_maps, core_ids=list(range(N_CORES)),
                               **kwargs)
    acc = np.zeros((B, DIM), dtype=np.float64)
    for r in res.results:
        acc += r["out"].astype(np.float64)
    out = acc.astype(np.float32).reshape(B, 1, DIM)
    if _trace:
        kernel._last_results = res
    return out


# revision 4
# speedup vs baseline: 1.0360x; 1.0360x over previous
"""Single-token-decode attention (b=16, h=32, d=128, kv=4096) on 8 NeuronCores.

Sharding: tensor-parallel over heads — 4 heads per core. Each core computes
q/k/v projections for its heads, attention over the full KV cache slice, and
its partial x @ wo contribution; the host sums the 8 partials.

Quantized-cache design (per-core HBM traffic ~75 MiB, vs 144 MiB bf16):
  - V cache: fp8 e3m4, natural [seq, hd] layout, consumed by PE (AV matmul
    streams fp8 moving against bf16 P stationary).
  - K cache, heads 0-1 ("DVE heads"): int8 with per-row fp32 scales
    (amax/127, 1/sqrt(d) folded in). Scores via DVE scalar_tensor_tensor
    with an IMMEDIATE scalar (AP scalars cost +62 ns/op); the row scales
    are applied afterwards as one [128, NT] tensor_mul per (b, ch, head).
  - K cache, heads 2-3 ("PE heads"): fp8 e3m4, host-transposed to
    [d, seq'] with seq' permuted so the PE score matmul (K^T-tile
    stationary [128d x 128seq], q column moving) lands S in the same
    [128(p), NT] seq-partition layout the DVE heads and V tiles use
    (s_local = p*16 + t).
  - wq/wo bf16; wk/wv fp8 e3m4 with x64 host prescale (they only affect
    the single patched token).

DMA packing: the four per-(b, chunk) streams (K-int8, scales, K^T-fp8,
V-fp8) are host-packed into ONE blob with 16512 contiguous bytes per
partition, loaded with a single DMA and consumed through bitcast views —
small/strided loads were costing ~25% of effective HBM bandwidth.
Weights are host-packed to [8, 128, 2048] so each partition is one
contiguous run. Exp runs on ACT (SBUF for DVE heads, PSUM for PE heads)
with per-chunk row-sum accumulation; normalization is one reciprocal +
broadcast-matmul at the end. The stale seq-4095 row is patched on-device
with dtype-converting ACT copies DMA'd into the blob views.
"""

import ml_dtypes
import numpy as np

import concourse.bass as bass
import concourse.mybir as mybir
import concourse.tile as tile
from concourse import bacc
from concourse.bass_utils import run_bass_kernel_spmd

N_CORES = 8
B = 16          # batch
H = 4           # heads per core
NHD = 2         # DVE (int8) heads: local 0,1
NHP = 2         # PE (fp8 K^T) heads: local 2,3
D = 128         # head dim
HD = H * D      # 512
DIM = 4096
SEQ = 4096
CH = 2048       # seq chunk per round
NT = CH // 128  # seq tiles per chunk (16)
NCH = SEQ // CH  # 2
NPAIR = H * B   # (head, batch) pairs per core
SCALE = float(1.0 / np.sqrt(np.float32(D)))
KPATCH_C = 8.0  # fixed int8 scale for the on-device patched K row
WKV_PRE = 64.0  # host prescale on wk/wv so fp8 e3m4 stays in normal range
F32 = mybir.dt.float32
BF16 = mybir.dt.bfloat16
FP8 = mybir.dt.float8e3
I8 = mybir.dt.int8

# blob byte offsets (per partition)
OFF_K8 = 0                      # int8 [NT, 256]            -> 4096 B
OFF_KS = 4096                   # f32  [NT, 2]              -> 128 B
OFF_KTP = 4224                  # fp8  [2, 2048]            -> 4096 B
OFF_V = 8320                    # fp8  [NT, 512]            -> 8192 B
BLOB_B = 16512

_nc_cache = {}


def _build_nc():
    if "nc" in _nc_cache:
        return _nc_cache["nc"]
    nc = bacc.Bacc("TRN2", target_bir_lowering=False, debug=False,
                   num_devices=N_CORES)

    xT = nc.dram_tensor("xT", [128, DIM // 128, B], BF16, kind="ExternalInput").ap()
    wq = nc.dram_tensor("wq", [8, 128, 2048], BF16, kind="ExternalInput").ap()
    wk = nc.dram_tensor("wk", [8, 128, 2048], FP8, kind="ExternalInput").ap()
    wv = nc.dram_tensor("wv", [8, 128, 2048], FP8, kind="ExternalInput").ap()
    wo = nc.dram_tensor("wo", [8, 128, 2048], BF16, kind="ExternalInput").ap()
    blob = nc.dram_tensor("blob", [B, NCH, 128, BLOB_B], I8,
                          kind="ExternalInput").ap()
    cos16 = nc.dram_tensor("cos16", [B, D // 2], F32, kind="ExternalInput").ap()
    id4 = nc.dram_tensor("id4", [H, H], F32, kind="ExternalInput").ap()
    id16 = nc.dram_tensor("id16", [B, B], F32, kind="ExternalInput").ap()
    bcsel = nc.dram_tensor("bcsel", [B, B * 128], BF16,
                           kind="ExternalInput").ap()
    sin16 = nc.dram_tensor("sin16", [B, D // 2], F32, kind="ExternalInput").ap()
    out = nc.dram_tensor("out", [B, DIM], F32, kind="ExternalOutput").ap()

    batch_order = list(range(1, B)) + [0]
    first_b = batch_order[0]

    def load_blob(pool, b, ch):
        t = pool.tile([128, BLOB_B], I8)
        nc.sync.dma_start(out=t, in_=blob[b, ch])
        return t

    def blob_views(bl):
        kt = bl[:, OFF_K8:OFF_KS].rearrange("p (t n) -> p t n", t=NT)
        kst = bl[:, OFF_KS:OFF_KTP].bitcast(F32).rearrange(
            "p (t h) -> p t h", t=NT)
        ktpt = bl[:, OFF_KTP:OFF_V].bitcast(FP8).rearrange(
            "p (h c) -> p h c", h=NHP)
        vt = bl[:, OFF_V:BLOB_B].bitcast(FP8).rearrange(
            "p (t n) -> p t n", t=NT)
        return kt, kst, ktpt, vt

    def rotary(dst, src, t1, t2, cos_sb, sin_sb):
        rv = dst.rearrange("b (h i two) -> b h i two", h=H, two=2)
        sv = src.rearrange("b (h i two) -> b h i two", h=H, two=2)
        for h in range(H):
            e, o = sv[:, h, :, 0], sv[:, h, :, 1]
            nc.vector.tensor_mul(t1[:, h, :], e, cos_sb)
            nc.vector.tensor_mul(t2[:, h, :], o, sin_sb)
            nc.vector.tensor_sub(rv[:, h, :, 0], t1[:, h, :], t2[:, h, :])
            nc.vector.tensor_mul(t1[:, h, :], e, sin_sb)
            nc.vector.tensor_mul(t2[:, h, :], o, cos_sb)
            nc.vector.tensor_add(rv[:, h, :, 1], t1[:, h, :], t2[:, h, :])

    with tile.TileContext(nc) as tc:
        with (
            tc.tile_pool(name="singles", bufs=1) as singles,
            tc.tile_pool(name="blobpool", bufs=6) as blobpool,
            tc.tile_pool(name="spool", bufs=4) as spool,
            tc.tile_pool(name="ppool", bufs=6) as ppool,
            tc.tile_pool(name="scrp", bufs=2) as scrp,
            tc.tile_pool(name="rowpool", bufs=8) as rowpool,
            tc.tile_pool(name="ps_a", bufs=2, space="PSUM") as ps_a,
            tc.tile_pool(name="ps_av", bufs=2, space="PSUM") as ps_av,
            tc.tile_pool(name="ps_s", bufs=2, space="PSUM") as ps_s,
            tc.tile_pool(name="ps_tr", bufs=1, space="PSUM") as ps_tr,
            tc.tile_pool(name="ps_sum", bufs=1, space="PSUM") as ps_sum,
        ):
            with tc.tile_pool(name="wqpool", bufs=7) as wqpool, \
                 tc.tile_pool(name="wpool", bufs=2) as wpool:
                with nc.named_scope("p1_proj"):
                    # ---- tiny constants first ----
                    xT_sb = singles.tile([128, DIM // 128, B], BF16)
                    nc.sync.dma_start(out=xT_sb, in_=xT)
                    cos_sb = singles.tile([B, D // 2], F32)
                    nc.sync.dma_start(out=cos_sb, in_=cos16)
                    sin_sb = singles.tile([B, D // 2], F32)
                    nc.sync.dma_start(out=sin_sb, in_=sin16)
                    ones_sb = singles.tile([128, 1], F32)
                    nc.vector.memset(ones_sb, 1.0)
                    ones_row = singles.tile([1, 128], F32)
                    nc.vector.memset(ones_row, 1.0)
                    id4_sb = singles.tile([H, H], F32)
                    nc.sync.dma_start(out=id4_sb, in_=id4)
                    id16_sb = singles.tile([B, B], F32)
                    nc.sync.dma_start(out=id16_sb, in_=id16)
                    bcsel_sb = singles.tile([B, B * 128], BF16)
                    nc.sync.dma_start(out=bcsel_sb, in_=bcsel)

                    # first batch's chunk tensors before the weights so the
                    # S stream can start as early as possible
                    pre_bl0 = load_blob(blobpool, first_b, 0)

                    # PE warm-up: dummy transposes ramp the clock while
                    # the wq stream arrives
                    warm_ps = ps_a.tile([B, B], F32, name="warm", tag="work")
                    for _ in range(40):
                        nc.tensor.transpose(warm_ps, id16_sb, id16_sb)

                    # ---- q projection (needs only wq) ----
                    qrow_sb = singles.tile([B, HD], F32)
                    proj_ps = ps_a.tile([B, HD], F32, name="proj_q",
                                        tag="work")
                    for ci in range(8):
                        wt = wqpool.tile([128, 2048], BF16, tag="w")
                        nc.sync.dma_start(out=wt, in_=wq[ci])
                        wt_v = wt.rearrange("p (s n) -> p s n", s=4)
                        for s in range(4):
                            ktile = ci * 4 + s
                            nc.tensor.matmul(proj_ps, xT_sb[:, ktile, :],
                                             wt_v[:, s, :], start=(ktile == 0),
                                             stop=(ktile == 31))
                    nc.scalar.copy(out=qrow_sb, in_=proj_ps)

                    # q rotary, then a bf16 copy for the broadcast matmuls
                    qrot_sb = singles.tile([B, HD], F32, name="rot_q")
                    t1 = singles.tile([B, H, D // 2], F32, name="t1")
                    t2 = singles.tile([B, H, D // 2], F32, name="t2")
                    rotary(qrot_sb, qrow_sb, t1, t2, cos_sb, sin_sb)
                    qrot_bf = singles.tile([B, HD], BF16, name="qrot_bf")
                    nc.scalar.copy(out=qrot_bf, in_=qrot_sb)

                    # DVE-head q broadcasts via one-hot selector matmuls
                    qbc_all = []
                    for b in range(B):
                        qbc_ps = ps_a.tile([128, NHD * D], F32, name="qbc_ps",
                                           tag="work")
                        nc.tensor.matmul(qbc_ps,
                                         bcsel_sb[:, b * 128:(b + 1) * 128],
                                         qrot_bf[:, :NHD * D],
                                         start=True, stop=True)
                        qbc_sb = singles.tile([128, NHD * D], BF16,
                                              name=f"qbc{b}")
                        nc.scalar.copy(out=qbc_sb, in_=qbc_ps)
                        qbc_all.append(qbc_sb)

                    # PE-head q columns: transpose qrot rows -> [d, b] columns
                    qT_sb = singles.tile([128, NHP, B], BF16, name="qT_sb")
                    for i in range(NHP):
                        h = NHD + i
                        qT_ps = ps_a.tile([128, B], F32, name="qT_ps",
                                          tag="work")
                        nc.tensor.transpose(qT_ps,
                                            qrot_sb[:, h * D:(h + 1) * D],
                                            id16_sb)
                        nc.scalar.copy(out=qT_sb[:, i, :], in_=qT_ps)

                    # first batch's chunk-1 before the k/v weights
                    pre_bl1 = load_blob(blobpool, first_b, 1)

                    # ---- k/v projections (fp8 weights, x64 prescaled) ----
                    krow_sb = singles.tile([B, HD], F32)
                    vnew_sb = singles.tile([B, HD], F32)
                    for w_d, row_sb in ((wk, krow_sb), (wv, vnew_sb)):
                        proj_ps = ps_a.tile([B, HD], F32, name="proj_kv",
                                            tag="work")
                        for ci in range(8):
                            wt = wpool.tile([128, 2048], FP8, tag="w")
                            nc.sync.dma_start(out=wt, in_=w_d[ci])
                            wt_v = wt.rearrange("p (s n) -> p s n", s=4)
                            for s in range(4):
                                ktile = ci * 4 + s
                                nc.tensor.matmul(proj_ps, xT_sb[:, ktile, :],
                                                 wt_v[:, s, :],
                                                 start=(ktile == 0),
                                                 stop=(ktile == 31))
                        nc.scalar.mul(out=row_sb, in_=proj_ps,
                                      mul=1.0 / WKV_PRE)

                    # k rotary + patch rows in each cache dtype
                    krot_sb = singles.tile([B, HD], F32, name="rot_k")
                    rotary(krot_sb, krow_sb, t1, t2, cos_sb, sin_sb)
                    kpatch8 = singles.tile([B, NHD * D], I8, name="kpatch8")
                    nc.scalar.mul(out=kpatch8, in_=krot_sb[:, :NHD * D],
                                  mul=127.0 / KPATCH_C)
                    kTp_sb = singles.tile([128, NHP, B], FP8, name="kTp_sb")
                    for i in range(NHP):
                        h = NHD + i
                        kT_ps = ps_a.tile([128, B], F32, name="kT_ps",
                                          tag="work")
                        nc.tensor.transpose(kT_ps,
                                            krot_sb[:, h * D:(h + 1) * D],
                                            id16_sb)
                        nc.scalar.copy(out=kTp_sb[:, i, :], in_=kT_ps)
                    vpatch8 = singles.tile([B, HD], FP8, name="vpatch8")
                    nc.scalar.copy(out=vpatch8, in_=vnew_sb)

            with tc.tile_pool(name="outp", bufs=2) as outp:
                # ---- phase 2: attention ----
                O_sb = singles.tile([128, NPAIR], F32)
                sums2_ps = ps_sum.tile([1, NCH * NPAIR], F32)
                wot_tiles = [None] * 8

                for bi, b in enumerate(batch_order):
                  with nc.named_scope(f"p2_b{b:02d}"):
                    qbc_sb = qbc_all[b]
                    av_ps = ps_av.tile([H, 512], F32)
                    rows_pair = [rowpool.tile([128, NCH], F32,
                                              name="rows_pair")
                                 for _ in range(H)]
                    ptils = []
                    vts = []
                    for ch in range(NCH):
                        if b == first_b:
                            bl = pre_bl0 if ch == 0 else pre_bl1
                        else:
                            bl = load_blob(blobpool, b, ch)
                        kt, kst, ktpt, vt = blob_views(bl)
                        vts.append(vt)
                        if ch == NCH - 1:
                            # seq position 4095 holds stale cache: replace
                            # with the new token's rotated k / v rows
                            nc.scalar.dma_start(out=kt[127:128, NT - 1, :],
                                                in_=kpatch8[b:b + 1, :])
                            nc.scalar.dma_start(
                                out=ktpt[:, :, CH - 1:CH],
                                in_=kTp_sb[:, :, b:b + 1])
                            nc.scalar.dma_start(out=vt[127:128, NT - 1, :],
                                                in_=vpatch8[b:b + 1, :])

                        ptil = ppool.tile([128, NT, H], BF16)
                        ptils.append(ptil)
                        # DVE heads: int8 STT (immediate scalar), then one
                        # row-scale multiply
                        for hi in range(NHD):
                            s_sb = spool.tile([128, NT], F32, name="s_sb")
                            for t in range(NT):
                                scr = scrp.tile([128, D], BF16, name="scr")
                                nc.vector.scalar_tensor_tensor(
                                    out=scr,
                                    in0=kt[:, t, hi * D:(hi + 1) * D],
                                    scalar=1.0,
                                    in1=qbc_sb[:, hi * D:(hi + 1) * D],
                                    op0=mybir.AluOpType.mult,
                                    op1=mybir.AluOpType.mult,
                                    accum_out=s_sb[:, t:t + 1])
                            nc.vector.tensor_mul(s_sb, s_sb, kst[:, :, hi])
                            nc.scalar.activation(
                                out=ptil[:, :, hi], in_=s_sb,
                                func=mybir.ActivationFunctionType.Exp,
                                scale=1.0,
                                accum_out=rows_pair[hi][:, ch:ch + 1])
                        # PE heads: K^T-tile stationary x q column moving
                        s_ps = ps_s.tile([128, NT, NHP], F32, name="s_pe")
                        for i in range(NHP):
                            for t in range(NT):
                                nc.tensor.matmul(
                                    s_ps[:, t, i:i + 1],
                                    ktpt[:, i, t * 128:(t + 1) * 128],
                                    qT_sb[:, i, b:b + 1],
                                    start=True, stop=True)
                            hi = NHD + i
                            nc.scalar.activation(
                                out=ptil[:, :, hi], in_=s_ps[:, :, i],
                                func=mybir.ActivationFunctionType.Exp,
                                scale=SCALE,
                                accum_out=rows_pair[hi][:, ch:ch + 1])
                    # AV for both chunks after both S streams: PE never
                    # stalls on the exp of the chunk it just scored
                    for ch in range(NCH):
                        for t in range(NT):
                            nc.tensor.matmul(av_ps, ptils[ch][:, t, :],
                                             vts[ch][:, t, :],
                                             start=(ch == 0 and t == 0),
                                             stop=(ch == NCH - 1
                                                   and t == NT - 1))

                    if 8 <= bi < 16:
                        # spread the wo prefetch through the later stream
                        # (early DMA belongs to the blob pipeline)
                        wot = singles.tile([128, 2048], BF16,
                                           name=f"wot{bi}")
                        nc.sync.dma_start(out=wot, in_=wo[bi - 8])
                        wot_tiles[bi - 8] = wot

                    for hi in range(H):
                        pc = hi * B + b
                        nc.tensor.matmul(
                            sums2_ps[0:1, NCH * pc:NCH * pc + NCH],
                            ones_sb, rows_pair[hi], start=True, stop=True)

                    # extract block-diagonal strips of av_ps and transpose
                    # into O^T [d, pair] layout
                    av_sb = outp.tile([H, 512], F32, name="av_sb")
                    nc.scalar.copy(out=av_sb, in_=av_ps)
                    tr_ps = ps_tr.tile([128, H, H], F32)
                    for hi in range(H):
                        nc.tensor.transpose(tr_ps[:, hi, :],
                                            av_sb[:, hi * D:(hi + 1) * D],
                                            id4_sb)
                    ocols = O_sb.rearrange("p (h bb) -> p h bb", h=H)
                    for hi in range(H):
                        nc.scalar.copy(out=ocols[:, hi, b:b + 1],
                                       in_=tr_ps[:, hi, hi:hi + 1])

                # ---- phase 3: normalize + wo ----
                with nc.named_scope("p3_out"):
                    sums2_sb = singles.tile([1, NCH * NPAIR], F32)
                    nc.scalar.copy(out=sums2_sb, in_=sums2_ps)
                    totals_sb = singles.tile([1, NPAIR], F32)
                    nc.vector.tensor_reduce(
                        out=totals_sb,
                        in_=sums2_sb.rearrange("p (a b) -> p a b", b=NCH),
                        axis=mybir.AxisListType.X, op=mybir.AluOpType.add)
                    rsum_sb = singles.tile([1, NPAIR], F32)
                    nc.vector.reciprocal(out=rsum_sb, in_=totals_sb)
                    bc_ps = ps_a.tile([128, NPAIR], F32, name="bc_ps",
                                      tag="work")
                    nc.tensor.matmul(bc_ps, ones_row, rsum_sb, start=True,
                                     stop=True)
                    on_sb = singles.tile([128, NPAIR], F32)
                    nc.vector.tensor_mul(on_sb, O_sb, bc_ps)
                    on_bf = singles.tile([128, NPAIR], BF16)
                    nc.scalar.copy(out=on_bf, in_=on_sb)

                    for nch in range(8):
                        wot_v = wot_tiles[nch].rearrange(
                            "p (k j) -> p k j", k=4)
                        wo_ps = ps_a.tile([B, 512], F32, name="wo_ps",
                                          tag="work")
                        for k in range(H):
                            nc.tensor.matmul(wo_ps,
                                             on_bf[:, k * B:(k + 1) * B],
                                             wot_v[:, k, :], start=(k == 0),
                                             stop=(k == H - 1))
                        wout_sb = outp.tile([B, 512], F32, name="wout_sb")
                        nc.scalar.copy(out=wout_sb, in_=wo_ps)
                        nc.scalar.dma_start(
                            out=out[:, nch * 512:(nch + 1) * 512],
                            in_=wout_sb)

    nc.compile()
    _nc_cache["nc"] = nc
    return nc


def _pack_w_in(w, cs):
    """[4096, 512] column slice -> [8, 128, 2048] with contiguous partitions."""
    return np.ascontiguousarray(
        w[:, cs].reshape(8, 4, 128, 512).transpose(0, 2, 1, 3).reshape(
            8, 128, 2048))


def _host_prep(x, wq, wk, wv, wo, cache_k, cache_v, freqs_cos, freqs_sin):
    f32 = np.float32
    bf = ml_dtypes.bfloat16
    f8 = ml_dtypes.float8_e3m4
    xT = np.ascontiguousarray(
        x.reshape(B, DIM // 128, 128).transpose(2, 1, 0)).astype(bf)

    cos = np.asarray(freqs_cos, dtype=f32).reshape(D // 2)
    sin = np.asarray(freqs_sin, dtype=f32).reshape(D // 2)
    cos16 = np.ascontiguousarray(np.broadcast_to(cos, (B, D // 2)), dtype=f32)
    sin16 = np.ascontiguousarray(np.broadcast_to(sin, (B, D // 2)), dtype=f32)

    bcsel = np.zeros((B, B * 128), dtype=bf)
    for b in range(B):
        bcsel[b, b * 128:(b + 1) * 128] = 1.0

    kc = np.asarray(cache_k, dtype=f32)   # [B, SEQ, 32, 128]
    vc = np.asarray(cache_v, dtype=f32)
    wq_bf = np.asarray(wq, dtype=f32).astype(bf)
    wk_f8 = (np.asarray(wk, dtype=f32) * WKV_PRE).astype(f8)
    wv_f8 = (np.asarray(wv, dtype=f32) * WKV_PRE).astype(f8)
    wo_bf = np.asarray(wo, dtype=f32).astype(bf)

    in_maps = []
    for c in range(N_CORES):
        hs = slice(H * c, H * (c + 1))
        cs = slice(HD * c, HD * (c + 1))
        k_c = kc[:, :, hs, :]              # [B, SEQ, 4, 128]
        v_c = vc[:, :, hs, :]

        # --- DVE heads: int8 + per-row scales ---
        kd = k_c[:, :, :NHD, :]
        amax = np.abs(kd).max(axis=-1)
        amax = np.maximum(amax, 1e-12)
        kq8 = np.clip(np.rint(kd * (127.0 / amax[..., None])),
                      -127, 127).astype(np.int8)
        # scales: [B, NCH, 128(p), NT(t), 2]; fold 1/sqrt(d)
        sc = (amax / (127.0 * np.sqrt(np.float32(D)))).astype(f32)
        sc = sc.reshape(B, NCH, 128, NT, NHD)
        sc[:, 1, 127, 15, :] = KPATCH_C / (127.0 * np.sqrt(np.float32(D)))

        # --- PE heads: fp8 K^T with seq' = (t, p) permuted columns ---
        kp = k_c[:, :, NHD:, :]
        kp = kp.reshape(B, NCH, 128, NT, NHP, D)
        kp = kp.transpose(0, 1, 5, 4, 3, 2)     # b, ch, d(p), h, t, p
        # per-partition layout [h, c=t*128+p]: need [b, ch, d, h, t, p]
        ktp_bytes = np.ascontiguousarray(
            kp.astype(f8)).view(np.int8).reshape(B, NCH, 128, 4096)

        k8_bytes = kq8.reshape(B, NCH, 128, NT, NHD * D).reshape(
            B, NCH, 128, 4096)
        ks_bytes = np.ascontiguousarray(sc).view(np.int8).reshape(
            B, NCH, 128, 128)
        v8_bytes = np.ascontiguousarray(
            v_c.reshape(B, NCH, 128, NT, HD).astype(f8)).view(
                np.int8).reshape(B, NCH, 128, 8192)

        blob = np.concatenate([k8_bytes, ks_bytes, ktp_bytes, v8_bytes],
                              axis=-1)
        assert blob.shape[-1] == BLOB_B

        in_maps.append({
            "xT": xT,
            "id4": np.eye(H, dtype=f32),
            "id16": np.eye(B, dtype=f32),
            "bcsel": bcsel,
            "wq": _pack_w_in(wq_bf, cs),
            "wk": _pack_w_in(wk_f8, cs),
            "wv": _pack_w_in(wv_f8, cs),
            "wo": np.ascontiguousarray(
                wo_bf[cs, :].reshape(4, 128, 8, 512).transpose(
                    2, 1, 0, 3).reshape(8, 128, 2048)),
            "blob": blob,
            "cos16": cos16,
            "sin16": sin16,
        })
    return in_maps


def kernel(x, wq, wk, wv, wo, cache_k, cache_v, freqs_cos, freqs_sin,
           start_pos, _trace=False, _trace_kwargs=None):
    assert int(start_pos) == SEQ - 1, "kernel is specialized for start_pos=4095"
    in_maps = _host_prep(np.asarray(x, dtype=np.float32), np.asarray(wq),
                         np.asarray(wk), np.asarray(wv), np.asarray(wo),
                         np.asarray(cache_k), np.asarray(cache_v),
                         np.asarray(freqs_cos), np.asarray(freqs_sin))
    nc = _build_nc()
    kwargs = {}
    if _trace:
        kwargs["trace"] = True
        if _trace_kwargs:
            kwargs.update(_trace_kwargs)
    res = run_bass_kernel_spmd(nc, in_maps, core_ids=list(range(N_CORES)),
                               **kwargs)
    acc = np.zeros((B, DIM), dtype=np.float64)
    for r in res.results:
        acc += r["out"].astype(np.float64)
    out = acc.astype(np.float32).reshape(B, 1, DIM)
    if _trace:
        kernel._last_results = res
    return out
